# revision 4
# baseline (speedup 1.0000x reference)
"""MHA trn2 kernel v3: fp16 value path (accuracy), wide fp16 exp, DVE mask
multiply, transposed AV accumulator, PSUM-bank-aware packing.

Sharding: 8 cores = (batch b = core//4) x (head-group g = core%4, 3 heads).
Per-core host-prepared inputs (d = contraction index over D=768):
  qt, kt : fp16 [128, 4wc, 6c, 512]  x[p,wc,c,s] = X[b][wc*512+s, 128c+p]
  vt     : fp16 [128, 4tc, 6c, 512]  same layout as qt/kt (token chunks)
  wq, wk, wv : fp16 [128, 6c, 192]   w[p,c,j] = W.T[128c+p, hs[j]]
  wo     : fp16 [96, 2gr, 768]       wo[p,gr,o] = W_O.T[hs[96gr+p], o]
  maskt  : fp16 [128, 4qq, 16kt, 512] mask[b,0, qq*512+q, kt*128+p] (0/1)
  identh : fp16 [128, 128]           I (PE transposes)
Output: out fp16 [2048, 768] partial (host sums 4 group-partials per batch).

Per (qq, kt): PSUM[key128, 3h*512q] = k.q (fp16 matmuls, per-head slices of
one 3-bank tile); ACT exp(0.125*x) -> em_raw fp16 [128,1536]; DVE multiplies
the 0/1 mask per head -> em fp16.  AV (transposed): av[q128, qt, h*66+c] +=
em_slice.T @ [v|1]; col 64 = denominator.  Normalize: DVE reciprocal + Pool
per-partition tensor_scalar -> attn fp16 [tok, ch].  Phase 3: PE-transpose
attn -> attnT [ch, tok], O-projection, fp16 out.
"""

import numpy as np

import concourse.bass as bass
import concourse.bacc as bacc
import concourse.tile as tile
import concourse.mybir as mybir

F32 = mybir.dt.float32
FP16 = mybir.dt.float16
AF = mybir.ActivationFunctionType
ALU = mybir.AluOpType

D = 768
DK = 64
NH = 3
HD = NH * DK            # 192
S = 2048
ST = S // 128           # 16
QQ = S // 512           # 4
KC = D // 128           # 6


def build_mha_v3(n_cores=8, reps=1):
    nc = bacc.Bacc("TRN2", target_bir_lowering=False, debug=False,
                   num_devices=n_cores)

    qt_d = nc.dram_tensor("qt", [128, QQ, KC, 512], FP16, kind="ExternalInput")
    kt_d = nc.dram_tensor("kt", [128, QQ, KC, 512], FP16, kind="ExternalInput")
    vt_d = nc.dram_tensor("vt", [128, QQ, KC, 512], FP16, kind="ExternalInput")
    wq_d = nc.dram_tensor("wq", [128, KC, HD], FP16, kind="ExternalInput")
    wk_d = nc.dram_tensor("wk", [128, KC, HD], FP16, kind="ExternalInput")
    wv_d = nc.dram_tensor("wv", [128, KC, HD], FP16, kind="ExternalInput")
    wo_d = nc.dram_tensor("wo", [96, 2, D], FP16, kind="ExternalInput")
    mask_d = nc.dram_tensor("maskt", [128, QQ, ST, 512], FP16,
                            kind="ExternalInput")
    idh_d = nc.dram_tensor("identh", [128, 128], FP16, kind="ExternalInput")
    out_d = nc.dram_tensor("out", [S, D], FP16, kind="ExternalOutput")

    with tile.TileContext(nc) as tc:
      with (
            tc.tile_pool(name="consts", bufs=1) as cp,
            tc.tile_pool(name="big", bufs=1) as bigp,
            tc.tile_pool(name="raw", bufs=2) as rawp,
            tc.tile_pool(name="emr", bufs=3) as emrp,
            tc.tile_pool(name="em", bufs=19) as emp,
            tc.tile_pool(name="mq", bufs=2) as mqp,
            tc.tile_pool(name="rc", bufs=2) as rcp,
            tc.tile_pool(name="outs", bufs=4) as outp,
      ):
        for _rep in range(reps):
            # ---------- loads (SP queue; deadline order) ----------
            wq_sb = cp.tile([128, KC, HD], FP16, tag="wq", name="wq_sb")
            wk_sb = cp.tile([128, KC, HD], FP16, tag="wk", name="wk_sb")
            wv_sb = cp.tile([128, KC, HD], FP16, tag="wv", name="wv_sb")
            idh_sb = cp.tile([128, 128], FP16, tag="idh", name="idh_sb")
            wo_sb = cp.tile([96, 2, D], FP16, tag="wo", name="wo_sb")
            mq_tiles = {}

            def load_mask(qq):
                t = mqp.tile([128, ST, 512], FP16, tag="mq", name=f"mq{qq}")
                nc.gpsimd.dma_start(t[:], mask_d.ap()[:, qq])
                mq_tiles[qq] = t

            kraw = {}
            qraw = {}
            vraw = {}

            def load_raw(pool_tag, dst_map, src_d, wc, name, split=False):
                t = rawp.tile([128, KC, 512], FP16, tag=pool_tag,
                              bufs=(2 if pool_tag == "kraw" else 3),
                              name=f"{name}{wc}")
                if split:
                    nc.sync.dma_start(t[:, 0:3], src_d.ap()[:, wc, 0:3])
                    nc.sync.dma_start(t[:, 3:KC], src_d.ap()[:, wc, 3:KC])
                else:
                    nc.sync.dma_start(t[:], src_d.ap()[:, wc])
                dst_map[wc] = t

            nc.sync.dma_start(wk_sb[:], wk_d.ap())
            nc.sync.dma_start(wq_sb[:], wq_d.ap())
            load_raw("kraw", kraw, kt_d, 0, "kraw", split=True)
            load_raw("qraw", qraw, qt_d, 0, "qraw", split=True)
            load_mask(0)

            # warm the Exp table off the critical chain
            warm = cp.tile([1, 2], F32, tag="warm", name="warm")
            nc.vector.memset(warm[:], 0.0)
            nc.scalar.activation(warm[:], warm[:], AF.Exp, scale=1.0)
            # warm the PE p-state ramp with dep-free junk matmuls
            if _rep == 0:
                wpe = cp.tile([128, 512], FP16, tag="wpe", name="wpe")
                nc.gpsimd.memset(wpe[:], 0.0)
                with tc.tile_pool(name="wps", bufs=1, space="PSUM") as wps:
                    wp = wps.tile([128, 512], F32, tag="wp", name="wp")
                    for _w in range(7):
                        nc.tensor.matmul(wp[:], wpe[:, 0:128], wpe[:],
                                         start=True, stop=True)

            load_raw("kraw", kraw, kt_d, 1, "kraw")
            load_raw("qraw", qraw, qt_d, 1, "qraw")
            nc.sync.dma_start(wv_sb[:], wv_d.ap())
            load_raw("kraw", kraw, kt_d, 2, "kraw")
            load_raw("vraw", vraw, vt_d, 0, "vraw")
            load_raw("kraw", kraw, kt_d, 3, "kraw")
            load_raw("qraw", qraw, qt_d, 2, "qraw")
            load_raw("vraw", vraw, vt_d, 1, "vraw")
            load_raw("qraw", qraw, qt_d, 3, "qraw")
            load_mask(1)
            load_raw("vraw", vraw, vt_d, 2, "vraw")
            load_raw("vraw", vraw, vt_d, 3, "vraw")
            nc.sync.dma_start(idh_sb[:], idh_d.ap())
            nc.sync.dma_start(wo_sb[:], wo_d.ap())

            # projected q/k: [128, 512] (h0 p0-63, h1 p64-127) + [64, 512]
            # (h2) per chunk; scores read these directly via base-partition
            # slices (no rearrange).
            qpa = [bigp.tile([128, 512], FP16, tag=f"qpa{wc}",
                             name=f"qpa{wc}") for wc in range(QQ)]
            qpb = [bigp.tile([64, 512], FP16, tag=f"qpb{wc}",
                             name=f"qpb{wc}") for wc in range(QQ)]
            kpa = [bigp.tile([128, 512], FP16, tag=f"kpa{wc}",
                             name=f"kpa{wc}") for wc in range(QQ)]
            kpb = [bigp.tile([64, 512], FP16, tag=f"kpb{wc}",
                             name=f"kpb{wc}") for wc in range(QQ)]
            v16 = bigp.tile([128, ST, NH, 66], FP16, tag="v16", name="v16")
            nc.gpsimd.memset(v16[:], 1.0)
            attn_sb = bigp.tile([128, ST, HD], FP16, tag="attn",
                                name="attn_sb")
            attnT = [bigp.tile([96, S], FP16, tag=f"attnT{g}",
                               name=f"attnT{g}") for g in range(2)]

            # ---------- helpers ----------
            proj_state = {}

            def proj_part(raw_map, w_sb, pa, pb, wc, part, pjp, pname):
                key = (pname, wc)
                if part == 0:
                    proj_state[key] = (
                        pjp.tile([128, 512], F32, tag="ka",
                                 name=f"ka{pname}{wc}"),
                        pjp.tile([64, 512], F32, tag="kb",
                                 name=f"kb{pname}{wc}"))
                ka, kb = proj_state[key]
                x = raw_map[wc]
                for kc in (2 * part, 2 * part + 1):
                    nc.tensor.matmul(ka[:], w_sb[:, kc, 0:128], x[:, kc, :],
                                     start=(kc == 0), stop=(kc == KC - 1))
                    nc.tensor.matmul(kb[:], w_sb[:, kc, 128:HD], x[:, kc, :],
                                     start=(kc == 0), stop=(kc == KC - 1))
                if part == 2:
                    nc.vector.tensor_copy(pa[wc][:], ka[:])
                    nc.vector.tensor_copy(pb[wc][:], kb[:])

            def proj_chunk(raw_map, w_sb, pa, pb, wc, pjp, pname):
                for part in range(3):
                    proj_part(raw_map, w_sb, pa, pb, wc, part, pjp, pname)

            def v_chunk(st, pjp):
                tag = "ka" if st % 2 == 0 else "kb"
                psv = pjp.tile([128, HD], F32, tag=tag, name=f"psv{st}")
                x = vraw[st // 4]
                o = (st % 4) * 128
                for kc in range(KC):
                    nc.tensor.matmul(psv[:], x[:, kc, o:o + 128],
                                     wv_sb[:, kc, :],
                                     start=(kc == 0), stop=(kc == KC - 1))
                nc.scalar.copy(
                    v16[:, st, :, 0:DK],
                    psv[:].rearrange("p (h d) -> p h d", h=NH))

            em_tiles = {}

            def qk_src(pa, pb, wc, h):
                if h == 0:
                    return pa[wc][0:64, :]
                if h == 1:
                    return pa[wc][64:128, :]
                return pb[wc][0:64, :]

            def scores_kt(qq, kt):
                sc = scp.tile([128, NH * 512], F32, tag="sc",
                              name=f"sc{qq}_{kt}")
                for h in range(NH):
                    lhsT = qk_src(kpa, kpb, kt // 4, h)[
                        :, (kt % 4) * 128:(kt % 4) * 128 + 128]
                    rhs = qk_src(qpa, qpb, qq, h)
                    nc.tensor.matmul(sc[:, h * 512:(h + 1) * 512],
                                     lhsT, rhs, start=True, stop=True)
                emr = emrp.tile([128, NH, 512], FP16, tag="emr",
                                name=f"emr{qq}_{kt}")
                nc.scalar.activation(emr[:], sc[:], AF.Exp, scale=0.125)
                em = emp.tile([128, NH, 512], FP16, tag="em",
                              name=f"em{qq}_{kt}")
                mq = mq_tiles[qq]
                nc.vector.tensor_tensor(
                    em[:], emr[:],
                    mq[:, kt:kt + 1, :].broadcast_to((128, NH, 512)),
                    ALU.mult)
                em_tiles[(qq, kt)] = em

            av_tiles = {}

            def av_step(qq, kt, avp):
                # av packed [128, 4qt, 256]; qt regions 1KB, (h,c) at h*66+c.
                # start=True zeroes the whole 2KB psum bank: only first
                # matmul per bank starts, only last per bank stops.
                if kt == 0:
                    av_tiles[qq] = avp.tile([128, QQ, 256], F32, tag="av",
                                            name=f"av{qq}")
                av = av_tiles[qq]
                em = em_tiles[(qq, kt)]
                for h in range(NH):
                    for qt in range(4):
                        nc.tensor.matmul(
                            av[:, qt, h * 66:h * 66 + 66],
                            em[:, h, qt * 128:qt * 128 + 128],
                            v16[:, kt, h, :],
                            start=(kt == 0 and h == 0 and qt in (0, 2)),
                            stop=(kt == ST - 1 and h == NH - 1
                                  and qt in (1, 3)))

            def normalize(qq):
                av = av_tiles[qq]
                rc = rcp.tile([128, QQ, NH, 1], F32, tag="rc", name=f"rc{qq}")
                for qt in range(4):
                    nc.vector.reciprocal(
                        rc[:, qt, :, 0],
                        av[:, qt, 0:NH * 66].rearrange(
                            "p (h c) -> p h c", h=NH)[:, :, DK])
                for qt in range(4):
                    st = qq * 4 + qt
                    nc.vector.tensor_tensor(
                        attn_sb[:, st, :].rearrange("p (h c) -> p h c", h=NH),
                        av[:, qt, 0:NH * 66].rearrange(
                            "p (h c) -> p h c", h=NH)[:, :, 0:DK],
                        rc[:, qt].broadcast_to((128, NH, DK)),
                        ALU.mult)

            # ---------- phases 1+2 ----------
            scp_ctx = tc.tile_pool(name="sc", bufs=2, space="PSUM")
            scp = scp_ctx.__enter__()
            K_PARTS = {0: (1, 0), 1: (1, 1), 2: (1, 2), 3: (2, 0),
                       4: (2, 1), 5: (2, 2), 6: (3, 0), 7: (3, 1), 8: (3, 2)}
            Q_PARTS = {9: [(1, 0)], 10: [(1, 1)], 11: [(1, 2)],
                       12: [(2, 0), (2, 1)], 13: [(2, 2), (3, 0)],
                       14: [(3, 1), (3, 2)]}
            V_CHUNKS = {(0, 6): [0], (0, 7): [1], (0, 8): [2], (0, 9): [3],
                        (0, 10): [4], (0, 11): [5], (0, 12): [6, 7],
                        (0, 13): [8, 9], (0, 14): [10, 11, 12],
                        (0, 15): [13, 14, 15]}

            pjp_ctx = tc.tile_pool(name="projps", bufs=1, space="PSUM")
            pjp = pjp_ctx.__enter__()
            avp_ctx = None
            avp = None

            proj_chunk(kraw, wk_sb, kpa, kpb, 0, pjp, "k")
            proj_chunk(qraw, wq_sb, qpa, qpb, 0, pjp, "q")

            for qq in range(QQ):
                if qq == 2:
                    load_mask(3)
                for kt in range(ST):
                    if (qq, kt) == (1, 2):
                        pjp_ctx.__exit__(None, None, None)
                        avp_ctx = tc.tile_pool(name="avps", bufs=1,
                                               space="PSUM")
                        avp = avp_ctx.__enter__()
                    if (qq, kt) == (1, 0):
                        load_mask(2)
                    scores_kt(qq, kt)
                    if qq == 0 and kt in K_PARTS:
                        wc, part = K_PARTS[kt]
                        proj_part(kraw, wk_sb, kpa, kpb, wc, part, pjp, "k")
                    if qq == 0 and kt in Q_PARTS:
                        for wc, part in Q_PARTS[kt]:
                            proj_part(qraw, wq_sb, qpa, qpb, wc, part,
                                      pjp, "q")
                    for st in V_CHUNKS.get((qq, kt), ()):
                        v_chunk(st, pjp)
                    if qq == 1 and kt >= 2:
                        av_step(0, kt - 2, avp)
                        if kt == ST - 1:
                            av_step(0, 14, avp)
                            av_step(0, 15, avp)
                            normalize(0)
                    elif qq == 2 and kt < 8:
                        av_step(1, 2 * kt, avp)
                        av_step(1, 2 * kt + 1, avp)
                        if kt == 7:
                            normalize(1)
                    elif qq == 3:
                        src_qq = 2 if kt < 8 else 3
                        base = 2 * kt if kt < 8 else 2 * (kt - 8)
                        av_step(src_qq, base, avp)
                        av_step(src_qq, base + 1, avp)
                        if kt == 7:
                            normalize(2)
            normalize(QQ - 1)
            avp_ctx.__exit__(None, None, None)
            scp_ctx.__exit__(None, None, None)

            # ---------- phase 3: transpose + O-projection ----------
            with (
                tc.tile_pool(name="pops", bufs=3, space="PSUM") as pop,
                tc.tile_pool(name="ptps", bufs=2, space="PSUM") as ptp,
            ):
                def copy_rot(i, dst, srcp):
                    if i % 2 == 0:
                        nc.vector.tensor_copy(dst, srcp)
                    else:
                        nc.scalar.copy(dst, srcp)

                for st in range(ST):
                    for g in range(2):
                        pt = ptp.tile([96, 128], FP16, tag="pt",
                                      name=f"pt{st}_{g}")
                        nc.tensor.transpose(
                            pt[:], attn_sb[:, st, g * 96:(g + 1) * 96],
                            idh_sb[:])
                        copy_rot(st * 2 + g,
                                 attnT[g][:, st * 128:(st + 1) * 128], pt[:])
                for st in range(ST):
                    po = pop.tile([128, D], F32, tag="po", name=f"po{st}")
                    for o0, o1 in ((0, 512), (512, D)):
                        nc.tensor.matmul(
                            po[:, o0:o1],
                            attnT[0][:, st * 128:(st + 1) * 128],
                            wo_sb[:, 0, o0:o1], start=True, stop=False)
                        nc.tensor.matmul(
                            po[:, o0:o1],
                            attnT[1][:, st * 128:(st + 1) * 128],
                            wo_sb[:, 1, o0:o1], start=False, stop=True)
                    ob = outp.tile([128, D], FP16, tag="ob", name=f"ob{st}")
                    copy_rot(st, ob[:], po[:])
                    nc.sync.dma_start(
                        out_d.ap()[st * 128:(st + 1) * 128, :], ob[:])

    nc.compile()
    return nc


# ---------------- host-side prep ----------------

def prep_core(Q, K, V, mask, W_Q, W_K, W_V, W_O, b, g):
    f16 = np.float16
    hs = slice(g * HD, (g + 1) * HD)

    def x_layout(X):
        # [128, 4, 6, 512]: x[p,wc,c,s] = X[wc*512+s, 128c+p]
        return np.ascontiguousarray(
            X.reshape(QQ, 512, KC, 128).transpose(3, 0, 2, 1)).astype(f16)

    def w_layout(W):
        return np.ascontiguousarray(
            W[hs, :].T.reshape(KC, 128, HD).transpose(1, 0, 2)).astype(f16)

    wo = np.ascontiguousarray(
        W_O.T[hs, :].reshape(2, 96, D).transpose(1, 0, 2)).astype(f16)

    # maskt[p, qq, kt, q] = mask[qq*512+q, kt*128+p]
    mt = np.ascontiguousarray(
        mask.T.reshape(ST, 128, QQ, 512).transpose(1, 2, 0, 3)).astype(f16)

    identh = np.eye(128, dtype=f16)

    return {
        "qt": x_layout(Q), "kt": x_layout(K), "vt": x_layout(V),
        "wq": w_layout(W_Q), "wk": w_layout(W_K), "wv": w_layout(W_V),
        "wo": wo, "maskt": mt, "identh": identh,
    }


def make_in_maps(Q, K, V, mask, W_Q, W_K, W_V, W_O, n_cores=8):
    in_maps = []
    for c in range(n_cores):
        b, g = divmod(c, 4)
        in_maps.append(prep_core(Q[b], K[b], V[b], mask[b, 0],
                                 W_Q, W_K, W_V, W_O, b, g))
    return in_maps


def combine_outputs(partials):
    ps = [p.astype(np.float32) for p in partials]
    b0 = ps[0] + ps[1] + ps[2] + ps[3]
    b1 = ps[4] + ps[5] + ps[6] + ps[7]
    return np.stack([b0, b1])


_NC_CACHE = {}


def _get_nc(reps=1):
    key = ("v3", reps)
    if key not in _NC_CACHE:
        _NC_CACHE[key] = build_mha_v3(n_cores=8, reps=reps)
    return _NC_CACHE[key]


def kernel(Q, K, V, mask, W_Q, W_K, W_V, W_O, _reps=1):
    from concourse.bass_utils import run_bass_kernel_spmd
    nc = _get_nc(_reps)
    in_maps = make_in_maps(
        np.asarray(Q, np.float32), np.asarray(K, np.float32),
        np.asarray(V, np.float32), np.asarray(mask),
        np.asarray(W_Q, np.float32), np.asarray(W_K, np.float32),
        np.asarray(W_V, np.float32), np.asarray(W_O, np.float32))
    res = run_bass_kernel_spmd(nc, in_maps, core_ids=list(range(8)))
    out = combine_outputs([res.results[c]["out"] for c in range(8)])
    return out.astype(np.float32)
